# revision 17
# baseline (speedup 1.0000x reference)
"""DequantingLinear Trainium2 kernel.

y = x @ W^T + b where W = (w_q - 128) * w_scales (GGML Q8_0-style, block=32),
b = (b_q - 128) * b_scales.

Sharding: column-parallel over out_features across 8 cores (1536 rows of W
per core, 18.9 MB of int32 codes each — the HBM-bound stream).

Dataflow per core, per 128-row o-tile:
  1. w_q streams in as 12 x 1.5 MB contiguous tiles, alternating between the
     SP HWDGE ring and the GpSimd SWDGE ring so each ring's per-transfer
     completion-receipt tail (~1us) hides behind the other ring's stream.
  2. DVE dequantizes with one fused scalar_tensor_tensor per half tile:
     wp = (w_q - 128) * scales -> fp16 (scales broadcast along the free dim
     with a step-0 AP).
  3. PE transposes wp 128x128 chunks (is_transpose matmul vs identity) into
     [128,1024] fp16 PSUM banks; ACT evacuates each bank in one copy.
  4. PE accumulates yT[128, 64] = sum_j wpt_j^T-as-lhsT @ xt_j — the
     transposed W chunk is the STATIONARY operand and the (tiny) activation
     x is the moving one, so each matmul streams only N=64 rows instead of
     N=128 — half the PE matmul time of the y-form.  The bias adds via one
     K=1 matmul (bias_row ⊗ ones-vector) into the same PSUM accumulator.
  5. yT shard [128, 12*64] fp32 DMAs out; the host transposes/concatenates.

x / scales / bias / identity are host-prepared into layouts whose DMA is
contiguous per partition (no strided descriptor sprays); w_q streams through
the device untouched.

Toolchain quirks handled (see _strip_self_waits and _patch_drain_split):
every ISA instruction encodes at most ONE semaphore wait; cheap same-engine
absorber ops take the DMA/slot-release waits up front, a post-pass drops
provably redundant waits, and the kernel-tail drain's global-clock waits are
pre-spread across SP nops.
"""

import sys

import numpy as np

for _p in ("/opt/trn_rl_repo", "/root/.axon_site/_ro/trn_rl_repo"):
    if _p not in sys.path:
        sys.path.append(_p)

B = 64          # batch (x is [64, 1, 3072])
IN = 3072       # in_features
OUT = 12288     # out_features
BLOCK = 32      # quant block
NB = IN // BLOCK            # 96 blocks per row
NCORES = 8
OSH = OUT // NCORES         # 1536 out features per core
OT = OSH // 128             # 12 o-tiles of 128 rows per core
KT = IN // 128              # 24 contraction tiles
NG = 3                      # k-chunk groups of 8 per o-tile

_CACHE: dict = {}


def _patch_drain_split():
    """The TRN2 ISA gives every instruction exactly ONE inline wait slot;
    Tile's kernel-tail drain asks for the whole global clock (~11 sems) on a
    single instruction, which walrus sometimes refuses ("Too many sync wait
    commands").  Pre-spread those waits across one SP nop per semaphore; the
    drain's own waits then elide via the SP engine clock."""
    from concourse import tile as tile_mod

    if getattr(tile_mod.TileContext, "_drain_split_patched", False):
        return
    from concourse.vector_clock import ScopedClock, VectorClock

    orig = tile_mod.TileContext._drain_and_barrier

    def patched(self, tick_clock, wait_clock):
        gvc = tick_clock.global_clock
        n = len(gvc)
        for p in range(n):
            t = gvc[p]
            if t <= 0:
                continue
            vc = VectorClock([0] * n)
            vc.require_at_least(p, t)
            nop = self.nc.sync.nop(hint="drain_wait_split", nofuse=True)
            wait_clock.add_sem_waits(nop.ins, ScopedClock({None: vc}))
        return orig(self, tick_clock, wait_clock)

    tile_mod.TileContext._drain_and_barrier = patched
    tile_mod.TileContext._drain_split_patched = True


def _build_nc():
    import concourse.bass as bass
    import concourse.mybir as mybir
    from concourse.tile import TileContext
    from contextlib import ExitStack

    _patch_drain_split()

    f32 = mybir.dt.float32
    i32 = mybir.dt.int32
    f16 = mybir.dt.float16

    nc = bass.Bass()
    wq = nc.declare_dram_parameter("wq", [OSH, IN], i32, isOutput=False)
    # swt[p, t*96+k] = w_scales[t*128+p, k]  (host-prepared, contiguous DMA)
    swt = nc.declare_dram_parameter("swt", [128, OT * NB], f32, isOutput=False)
    # xt[p, j*64+b] = x[b, j*128+p]
    xt = nc.declare_dram_parameter("xt", [128, KT * B], f16, isOutput=False)
    bq = nc.declare_dram_parameter("bq", [1, OSH], i32, isOutput=False)
    bs = nc.declare_dram_parameter("bs", [1, OSH // BLOCK], f32, isOutput=False)
    ident = nc.declare_dram_parameter("ident", [128, 128], f16, isOutput=False)
    # y[p, t*64+b] = y_full[b, c*1536 + t*128 + p]
    y = nc.declare_dram_parameter("y", [128, OT * B], f32, isOutput=True)

    with TileContext(nc) as tc, ExitStack() as ctx:
        const = ctx.enter_context(tc.tile_pool(name="const", bufs=1))
        wq_pool = ctx.enter_context(tc.tile_pool(name="wq", bufs=5))
        wp_pool = ctx.enter_context(tc.tile_pool(name="wp", bufs=5))
        wpt_pool = ctx.enter_context(tc.tile_pool(name="wpt", bufs=4))
        ysb_pool = ctx.enter_context(tc.tile_pool(name="ysb", bufs=1))
        pt_pool = ctx.enter_context(tc.tile_pool(name="pt", bufs=5, space="PSUM"))
        yt_pool = ctx.enter_context(tc.tile_pool(name="yt", bufs=2, space="PSUM"))
        sc_pool = ctx.enter_context(tc.tile_pool(name="sc", bufs=1, space="PSUM"))

        # --- wq stream: 12 single-tile 1.5 MB transfers, all on the single
        # SP HWDGE ring (a split across two rings measurably collapses
        # aggregate HBM bandwidth from ~430 to ~320 GB/s; per-transfer
        # receipt tails pipeline fine within one ring).  Per-tile transfers
        # keep the dequant granularity at one tile — a 3 MB pair stalls the
        # pair's first dequant on the second tile's bytes (~2-3us/pair).
        # Tiles 0 and 11 stream as two 0.75 MB halves for an earlier first
        # dequant / shorter drain tail.
        wq_view = {}
        for t in range(OT):
            wq_s = wq_pool.tile([128, IN], i32)
            if t in (0, OT - 1):
                for hh in range(2):
                    sl = slice(hh * IN // 2, (hh + 1) * IN // 2)
                    nc.sync.dma_start(wq_s[:, sl], wq[128 * t : 128 * (t + 1), sl])
            else:
                nc.sync.dma_start(wq_s[:], wq[128 * t : 128 * (t + 1), :])
            wq_view[t] = wq_s

        # --- constants / small inputs on the GpSimd SWDGE ring (so they do
        # not interrupt the wq stream); bias codes first (they feed the
        # GpSimd-side bias dequant), then scales (gate the first dequant) ---
        bq_sb = const.tile([1, OSH], i32)
        nc.gpsimd.dma_start(bq_sb[:], bq[:, :])
        bs_sb = const.tile([1, OSH // BLOCK], f32)
        nc.gpsimd.dma_start(bs_sb[:], bs[:, :])
        s_all = const.tile([128, OT * NB], f32)
        nc.gpsimd.dma_start(s_all[:], swt[:, :])
        id_sb = const.tile([128, 128], f16)
        nc.gpsimd.dma_start(id_sb[:], ident[:, :])
        xt_sb = const.tile([128, KT * B], f16)
        nc.gpsimd.dma_start(xt_sb[:], xt[:, :])

        # Wait-absorber scratch: the TensorScalarPtr(STT) ISA struct carries
        # at most ONE sync wait; before each STT we touch its input/output
        # tiles with cheap DVE ops so DMA-completion / slot-release waits
        # attach to those instead.
        scr = const.tile([1, 64], f32)
        ones = const.tile([1, B], f16)
        nc.vector.memset(ones[0:1, :], 1.0)

        # bias dequant (single partition, 1536 elems).  Runs first on DVE:
        # its inputs land ~2us before the scales that gate the first weight
        # dequant, so this is off the critical path.
        bias_sb = const.tile([1, OSH], f16)
        nc.vector.tensor_copy(scr[0:1, 0:1], bq_sb[0:1, 0:1])
        nc.vector.tensor_copy(scr[0:1, 1:2], bs_sb[0:1, 0:1])
        nc.vector.scalar_tensor_tensor(
            bias_sb[:].rearrange("o (k j) -> o k j", j=BLOCK),
            bq_sb[:].rearrange("o (k j) -> o k j", j=BLOCK),
            128.0,
            bs_sb[:].unsqueeze(2).broadcast_to([1, OSH // BLOCK, BLOCK]),
            mybir.AluOpType.subtract,
            mybir.AluOpType.mult,
        )

        y_sb = ysb_pool.tile([128, OT * B], f32)

        # PE wait-absorbers: the matmul LW ISA struct also carries at most
        # one sync wait.  Touch each constant input with a tiny matmul so
        # one-time DMA waits spread over separate PE instructions.  Only
        # id/xt gate the first transposes; the bias/ones absorbers are
        # emitted later (before tile 0's bias matmul) so the bias-path DMAs
        # don't delay PE start.
        scrap = sc_pool.tile([1, 8], f32, tag="sc")
        for i, src in enumerate((id_sb, xt_sb)):
            nc.tensor.matmul(
                scrap[0:1, i : i + 1], src[:, 0:1], src[:, 0:1],
                start=True, stop=True,
            )

        for t in range(OT):
            wq_t = wq_view[t]
            wp_t = wp_pool.tile([128, IN], f16)
            nc.vector.tensor_copy(scr[0:1, 4 + t : 5 + t], wq_t[0:1, 0:1])
            nc.vector.memset(wp_t[0:1, 0:1], 0.0)

            yt = yt_pool.tile([128, B], f32)
            # Per k-chunk group of 8 ([128, 1024]): one dequant STT third,
            # 8 transposes into one fp16 PSUM bank, ONE ACT evacuation, then
            # 8 matmuls with the evacuated W^T chunk stationary and xt
            # moving (N=64).  Group-aligned dequant keeps the transpose /
            # evac / matmul chain chasing each dequant chunk closely.
            for g in range(NG):
                sl = slice(g * 1024, (g + 1) * 1024)
                nc.vector.scalar_tensor_tensor(
                    wp_t[:, sl].rearrange("p (k j) -> p k j", j=BLOCK),
                    wq_t[:, sl].rearrange("p (k j) -> p k j", j=BLOCK),
                    128.0,
                    s_all[:, t * NB + g * NB // NG : t * NB + (g + 1) * NB // NG]
                    .unsqueeze(2)
                    .broadcast_to([128, NB // NG, BLOCK]),
                    mybir.AluOpType.subtract,
                    mybir.AluOpType.mult,
                )
                pt = pt_pool.tile([128, 1024], f16)
                for jj in range(8):
                    j = 8 * g + jj
                    nc.tensor.transpose(
                        pt[:, 128 * jj : 128 * (jj + 1)],
                        wp_t[:, 128 * j : 128 * (j + 1)],
                        id_sb[:],
                    )
                wpt = wpt_pool.tile([128, 1024], f16)
                nc.scalar.copy(wpt[:], pt[:])
                for jj in range(8):
                    j = 8 * g + jj
                    nc.tensor.matmul(
                        yt[:],
                        wpt[:, 128 * jj : 128 * (jj + 1)],
                        xt_sb[:, B * j : B * (j + 1)],
                        start=(j == 0),
                        stop=False,
                    )
            if t == 0:
                # bias/ones PE wait-absorbers, right before first use
                for i, src in enumerate((bias_sb, ones)):
                    nc.tensor.matmul(
                        scrap[0:1, 2 + i : 3 + i], src[0:1, 0:1], src[0:1, 0:1],
                        start=True, stop=True,
                    )
            # += bias via K=1 matmul: bias row (stationary) ⊗ ones (moving)
            nc.tensor.matmul(
                yt[:],
                bias_sb[0:1, 128 * t : 128 * (t + 1)],
                ones[0:1, :],
                start=False,
                stop=True,
            )
            nc.scalar.copy(y_sb[:, B * t : B * (t + 1)], yt[:])
            if t == OT // 2 - 1:
                # first half of the output leaves early so the final DMA is
                # half as long after the last tile completes
                nc.sync.dma_start(
                    y[:, : B * OT // 2], y_sb[:, : B * OT // 2]
                )

        nc.sync.dma_start(
            y[:, B * OT // 2 :], y_sb[:, B * OT // 2 :]
        )

    _strip_self_waits(nc, mybir)
    return nc


# NOTE: Pool (GPSIMD) is deliberately absent — it is 8 parallel Q7 cores, so
# same-engine ordering does NOT hold there and its self-waits are load-bearing.
_ENGINE_SEM_PREFIX = {
    "PE": "PE_",
    "DVE": "DVE_",
    "Activation": "Activation_",
    "SP": "SP_",
}


def _strip_self_waits(nc, mybir):
    """Several TRN2 ISA instruction structs encode at most ONE sync wait
    (walrus: "Too many sync wait commands").  Two classes of Tile-emitted
    waits are redundant and safe to drop from instructions carrying >=2:

    1. Self-engine waits: an engine completes its own instructions in order.
    2. DMAHW/DMASW waits on the wq streaming loads: the slot's previous DMA
       was fully consumed by the DVE dequant before the slot-release (DVE)
       wait tick, so the DVE wait transitively covers the DMA-WAW ordering
       (Tile's per-proc vector clock does not track transitivity).
    """
    fn = nc.m.functions[0]
    observed: dict = {}
    for b in fn.blocks:
        for inst in b.instructions:
            si = inst.sync_info
            if si is None or not si.on_wait:
                continue
            eng = str(inst.engine)
            if len(si.on_wait) < 2:
                for w in si.on_wait:
                    k = (eng, w.ant_name)
                    observed[k] = max(observed.get(k, 0), w.wait_value)
                continue
            keep = [
                w
                for w in si.on_wait
                if observed.get((eng, w.ant_name), 0) < w.wait_value
            ]
            pref = _ENGINE_SEM_PREFIX.get(str(inst.engine).split(".")[-1])
            if pref is not None:
                keep = [w for w in keep if not w.ant_name.startswith(pref)]
            if len(keep) >= 2 and type(inst).__name__ == "InstDMACopy":
                if any(
                    not w.ant_name.startswith(("DMAHW", "DMASW")) for w in keep
                ):
                    keep = [
                        w
                        for w in keep
                        if not w.ant_name.startswith(("DMAHW", "DMASW"))
                    ]
            for w in keep:
                k = (eng, w.ant_name)
                observed[k] = max(observed.get(k, 0), w.wait_value)
            if len(keep) != len(si.on_wait):
                inst.sync_info = mybir.SyncInfo(
                    on_wait=keep, on_update=si.on_update
                )


def _get_nc():
    if "nc" not in _CACHE:
        _CACHE["nc"] = _build_nc()
    return _CACHE["nc"]


def _make_in_maps(x, w_q, w_scales, b_q, b_scales):
    x2 = np.ascontiguousarray(x.reshape(B, IN), dtype=np.float32)
    # xt[p, j*64+b] = x[b, j*128+p]
    xt = np.ascontiguousarray(
        x2.T.reshape(KT, 128, B).transpose(1, 0, 2).reshape(128, KT * B)
    ).astype(np.float16)
    wq_full = np.ascontiguousarray(w_q.reshape(OUT, IN))  # int32 codes
    ws_full = np.ascontiguousarray(w_scales)              # [12288, 96]
    bq_full = np.ascontiguousarray(b_q.reshape(OUT))      # int32 codes
    bs_full = np.ascontiguousarray(b_scales)              # [384]
    ident = np.eye(128, dtype=np.float16)

    in_maps = []
    for c in range(NCORES):
        o0, o1 = c * OSH, (c + 1) * OSH
        ws_c = ws_full[o0:o1]  # [1536, 96]
        swt = np.ascontiguousarray(
            ws_c.reshape(OT, 128, NB).transpose(1, 0, 2).reshape(128, OT * NB)
        )
        in_maps.append(
            {
                "wq": np.ascontiguousarray(wq_full[o0:o1]),
                "swt": swt,
                "xt": xt,
                "bq": np.ascontiguousarray(bq_full[o0:o1]).reshape(1, OSH),
                "bs": np.ascontiguousarray(
                    bs_full[o0 // BLOCK : o1 // BLOCK]
                ).reshape(1, OSH // BLOCK),
                "ident": ident,
            }
        )
    return in_maps


def run_shards(x, w_q, w_scales, b_q, b_scales, trace=False):
    """Run the SPMD kernel; returns (y_full, BassKernelResults)."""
    from concourse.bass_utils import run_bass_kernel_spmd

    nc = _get_nc()
    in_maps = _make_in_maps(x, w_q, w_scales, b_q, b_scales)
    res = run_bass_kernel_spmd(
        nc, in_maps, core_ids=list(range(NCORES)), trace=trace
    )
    shards = []
    for c in range(NCORES):
        yt = np.asarray(res.results[c]["y"])  # [128, OT*B]
        shards.append(
            yt.reshape(128, OT, B).transpose(2, 1, 0).reshape(B, OSH)
        )
    y = np.concatenate(shards, axis=1).reshape(B, 1, OUT)
    return y, res


def kernel(**inputs):
    y, _ = run_shards(
        inputs["x"],
        inputs["w_q"],
        inputs["w_scales"],
        inputs["b_q"],
        inputs["b_scales"],
        trace=False,
    )
    return y.astype(np.float32)


# revision 21
# speedup vs baseline: 1.0088x; 1.0088x over previous
"""DequantingLinear Trainium2 kernel.

y = x @ W^T + b where W = (w_q - 128) * w_scales (GGML Q8_0-style, block=32),
b = (b_q - 128) * b_scales.

Sharding: column-parallel over out_features across 8 cores (1536 rows of W
per core, 18.9 MB of int32 codes each — the HBM-bound stream).

Dataflow per core, per 128-row o-tile:
  1. w_q streams in as 12 x 1.5 MB contiguous tiles, alternating between the
     SP HWDGE ring and the GpSimd SWDGE ring so each ring's per-transfer
     completion-receipt tail (~1us) hides behind the other ring's stream.
  2. DVE dequantizes with one fused scalar_tensor_tensor per half tile:
     wp = (w_q - 128) * scales -> fp16 (scales broadcast along the free dim
     with a step-0 AP).
  3. PE transposes wp 128x128 chunks (is_transpose matmul vs identity) into
     [128,1024] fp16 PSUM banks; ACT evacuates each bank in one copy.
  4. PE accumulates yT[128, 64] = sum_j wpt_j^T-as-lhsT @ xt_j — the
     transposed W chunk is the STATIONARY operand and the (tiny) activation
     x is the moving one, so each matmul streams only N=64 rows instead of
     N=128 — half the PE matmul time of the y-form.  The bias adds via one
     K=1 matmul (bias_row ⊗ ones-vector) into the same PSUM accumulator.
  5. yT shard [128, 12*64] fp32 DMAs out; the host transposes/concatenates.

x / scales / bias / identity are host-prepared into layouts whose DMA is
contiguous per partition (no strided descriptor sprays); w_q streams through
the device untouched.

Toolchain quirks handled (see _strip_self_waits and _patch_drain_split):
every ISA instruction encodes at most ONE semaphore wait; cheap same-engine
absorber ops take the DMA/slot-release waits up front, a post-pass drops
provably redundant waits, and the kernel-tail drain's global-clock waits are
pre-spread across SP nops.
"""

import sys

import numpy as np

for _p in ("/opt/trn_rl_repo", "/root/.axon_site/_ro/trn_rl_repo"):
    if _p not in sys.path:
        sys.path.append(_p)

B = 64          # batch (x is [64, 1, 3072])
IN = 3072       # in_features
OUT = 12288     # out_features
BLOCK = 32      # quant block
NB = IN // BLOCK            # 96 blocks per row
NCORES = 8
OSH = OUT // NCORES         # 1536 out features per core
OT = OSH // 128             # 12 o-tiles of 128 rows per core
KT = IN // 128              # 24 contraction tiles
NG = 3                      # k-chunk groups of 8 per o-tile

_CACHE: dict = {}


def _patch_drain_split():
    """The TRN2 ISA gives every instruction exactly ONE inline wait slot;
    Tile's kernel-tail drain asks for the whole global clock (~11 sems) on a
    single instruction, which walrus sometimes refuses ("Too many sync wait
    commands").  Pre-spread those waits across one SP nop per semaphore; the
    drain's own waits then elide via the SP engine clock."""
    from concourse import tile as tile_mod

    if getattr(tile_mod.TileContext, "_drain_split_patched", False):
        return
    from concourse.vector_clock import ScopedClock, VectorClock

    orig = tile_mod.TileContext._drain_and_barrier

    def patched(self, tick_clock, wait_clock):
        gvc = tick_clock.global_clock
        n = len(gvc)
        for p in range(n):
            t = gvc[p]
            if t <= 0:
                continue
            vc = VectorClock([0] * n)
            vc.require_at_least(p, t)
            nop = self.nc.sync.nop(hint="drain_wait_split", nofuse=True)
            wait_clock.add_sem_waits(nop.ins, ScopedClock({None: vc}))
        return orig(self, tick_clock, wait_clock)

    tile_mod.TileContext._drain_and_barrier = patched
    tile_mod.TileContext._drain_split_patched = True


def _build_nc():
    import concourse.bass as bass
    import concourse.mybir as mybir
    from concourse.tile import TileContext
    from contextlib import ExitStack

    _patch_drain_split()

    f32 = mybir.dt.float32
    i32 = mybir.dt.int32
    f16 = mybir.dt.float16

    nc = bass.Bass()
    wq = nc.declare_dram_parameter("wq", [OSH, IN], i32, isOutput=False)
    # swt[p, t*96+k] = w_scales[t*128+p, k]  (host-prepared, contiguous DMA)
    swt = nc.declare_dram_parameter("swt", [128, OT * NB], f32, isOutput=False)
    # xt[p, j*64+b] = x[b, j*128+p]
    xt = nc.declare_dram_parameter("xt", [128, KT * B], f16, isOutput=False)
    bq = nc.declare_dram_parameter("bq", [1, OSH], i32, isOutput=False)
    bs = nc.declare_dram_parameter("bs", [1, OSH // BLOCK], f32, isOutput=False)
    ident = nc.declare_dram_parameter("ident", [128, 128], f16, isOutput=False)
    # y[p, t*64+b] = y_full[b, c*1536 + t*128 + p]
    y = nc.declare_dram_parameter("y", [128, OT * B], f32, isOutput=True)

    with TileContext(nc) as tc, ExitStack() as ctx:
        const = ctx.enter_context(tc.tile_pool(name="const", bufs=1))
        wq_pool = ctx.enter_context(tc.tile_pool(name="wq", bufs=7))
        wp_pool = ctx.enter_context(tc.tile_pool(name="wp", bufs=5))
        wpt_pool = ctx.enter_context(tc.tile_pool(name="wpt", bufs=4))
        ysb_pool = ctx.enter_context(tc.tile_pool(name="ysb", bufs=1))
        pt_pool = ctx.enter_context(tc.tile_pool(name="pt", bufs=5, space="PSUM"))
        yt_pool = ctx.enter_context(tc.tile_pool(name="yt", bufs=2, space="PSUM"))
        sc_pool = ctx.enter_context(tc.tile_pool(name="sc", bufs=1, space="PSUM"))

        # --- wq stream: 12 single-tile 1.5 MB transfers, all on the single
        # SP HWDGE ring (a split across two rings measurably collapses
        # aggregate HBM bandwidth from ~430 to ~320 GB/s; per-transfer
        # receipt tails pipeline fine within one ring).  Per-tile transfers
        # keep the dequant granularity at one tile — a 3 MB pair stalls the
        # pair's first dequant on the second tile's bytes (~2-3us/pair).
        # Tiles 0 and 11 stream as two 0.75 MB halves for an earlier first
        # dequant / shorter drain tail.
        wq_view = {}
        for t in range(OT):
            wq_s = wq_pool.tile([128, IN], i32)
            if t in (0, OT - 1):
                for hh in range(2):
                    sl = slice(hh * IN // 2, (hh + 1) * IN // 2)
                    nc.sync.dma_start(wq_s[:, sl], wq[128 * t : 128 * (t + 1), sl])
            else:
                nc.sync.dma_start(wq_s[:], wq[128 * t : 128 * (t + 1), :])
            wq_view[t] = wq_s

        # --- constants / small inputs on the GpSimd SWDGE ring (so they do
        # not interrupt the wq stream); scales FIRST — they gate the first
        # dequant ---
        s_all = const.tile([128, OT * NB], f32)
        nc.gpsimd.dma_start(s_all[:], swt[:, :])
        bq_sb = const.tile([1, OSH], i32)
        nc.gpsimd.dma_start(bq_sb[:], bq[:, :])
        bs_sb = const.tile([1, OSH // BLOCK], f32)
        nc.gpsimd.dma_start(bs_sb[:], bs[:, :])
        id_sb = const.tile([128, 128], f16)
        nc.gpsimd.dma_start(id_sb[:], ident[:, :])
        xt_sb = const.tile([128, KT * B], f16)
        nc.gpsimd.dma_start(xt_sb[:], xt[:, :])

        # Wait-absorber scratch: the TensorScalarPtr(STT) ISA struct carries
        # at most ONE sync wait; before each STT we touch its input/output
        # tiles with cheap DVE ops so DMA-completion / slot-release waits
        # attach to those instead.
        scr = const.tile([1, 64], f32)
        ones = const.tile([1, B], f16)
        nc.vector.memset(ones[0:1, :], 1.0)

        # bias dequant happens per o-tile inside the loop ([1, 128] STT,
        # ~200ns) so it never blocks the in-order DVE weight-dequant stream.
        bias_sb = const.tile([1, OSH], f16)

        y_sb = ysb_pool.tile([128, OT * B], f32)

        # PE wait-absorbers: the matmul LW ISA struct also carries at most
        # one sync wait.  Touch each constant input with a tiny matmul so
        # one-time DMA waits spread over separate PE instructions.  Only
        # id/xt gate the first transposes; the bias/ones absorbers are
        # emitted later (before tile 0's bias matmul) so the bias-path DMAs
        # don't delay PE start.
        scrap = sc_pool.tile([1, 8], f32, tag="sc")
        for i, src in enumerate((id_sb, xt_sb)):
            nc.tensor.matmul(
                scrap[0:1, i : i + 1], src[:, 0:1], src[:, 0:1],
                start=True, stop=True,
            )

        for t in range(OT):
            wq_t = wq_view[t]
            wp_t = wp_pool.tile([128, IN], f16)
            if t == 0:
                # one-time absorbers for the bias-path DMA waits
                nc.vector.tensor_copy(scr[0:1, 0:1], bq_sb[0:1, 0:1])
                nc.vector.tensor_copy(scr[0:1, 1:2], bs_sb[0:1, 0:1])
            nc.vector.tensor_copy(scr[0:1, 4 + t : 5 + t], wq_t[0:1, 0:1])
            nc.vector.memset(wp_t[0:1, 0:1], 0.0)
            # per-tile bias dequant: [1, 128] slice, 4 scale blocks
            nc.vector.scalar_tensor_tensor(
                bias_sb[0:1, 128 * t : 128 * (t + 1)].rearrange(
                    "o (k j) -> o k j", j=BLOCK
                ),
                bq_sb[0:1, 128 * t : 128 * (t + 1)].rearrange(
                    "o (k j) -> o k j", j=BLOCK
                ),
                128.0,
                bs_sb[0:1, 4 * t : 4 * (t + 1)]
                .unsqueeze(2)
                .broadcast_to([1, 4, BLOCK]),
                mybir.AluOpType.subtract,
                mybir.AluOpType.mult,
            )

            yt = yt_pool.tile([128, B], f32)
            # Per k-chunk group of 8 ([128, 1024]): one dequant STT third,
            # 8 transposes into one fp16 PSUM bank, ONE ACT evacuation, then
            # 8 matmuls with the evacuated W^T chunk stationary and xt
            # moving (N=64).  Group-aligned dequant keeps the transpose /
            # evac / matmul chain chasing each dequant chunk closely.
            for g in range(NG):
                sl = slice(g * 1024, (g + 1) * 1024)
                nc.vector.scalar_tensor_tensor(
                    wp_t[:, sl].rearrange("p (k j) -> p k j", j=BLOCK),
                    wq_t[:, sl].rearrange("p (k j) -> p k j", j=BLOCK),
                    128.0,
                    s_all[:, t * NB + g * NB // NG : t * NB + (g + 1) * NB // NG]
                    .unsqueeze(2)
                    .broadcast_to([128, NB // NG, BLOCK]),
                    mybir.AluOpType.subtract,
                    mybir.AluOpType.mult,
                )
                pt = pt_pool.tile([128, 1024], f16)
                for jj in range(8):
                    j = 8 * g + jj
                    nc.tensor.transpose(
                        pt[:, 128 * jj : 128 * (jj + 1)],
                        wp_t[:, 128 * j : 128 * (j + 1)],
                        id_sb[:],
                    )
                wpt = wpt_pool.tile([128, 1024], f16)
                nc.scalar.copy(wpt[:], pt[:])
                for jj in range(8):
                    j = 8 * g + jj
                    nc.tensor.matmul(
                        yt[:],
                        wpt[:, 128 * jj : 128 * (jj + 1)],
                        xt_sb[:, B * j : B * (j + 1)],
                        start=(j == 0),
                        stop=False,
                    )
            if t == 0:
                # bias/ones PE wait-absorbers, right before first use
                for i, src in enumerate((bias_sb, ones)):
                    nc.tensor.matmul(
                        scrap[0:1, 2 + i : 3 + i], src[0:1, 0:1], src[0:1, 0:1],
                        start=True, stop=True,
                    )
            # += bias via K=1 matmul: bias row (stationary) ⊗ ones (moving)
            nc.tensor.matmul(
                yt[:],
                bias_sb[0:1, 128 * t : 128 * (t + 1)],
                ones[0:1, :],
                start=False,
                stop=True,
            )
            nc.scalar.copy(y_sb[:, B * t : B * (t + 1)], yt[:])
            if t == OT // 2 - 1:
                # first half of the output leaves early so the final DMA is
                # half as long after the last tile completes
                nc.sync.dma_start(
                    y[:, : B * OT // 2], y_sb[:, : B * OT // 2]
                )

        nc.sync.dma_start(
            y[:, B * OT // 2 :], y_sb[:, B * OT // 2 :]
        )

    _strip_self_waits(nc, mybir)
    return nc


# NOTE: Pool (GPSIMD) is deliberately absent — it is 8 parallel Q7 cores, so
# same-engine ordering does NOT hold there and its self-waits are load-bearing.
_ENGINE_SEM_PREFIX = {
    "PE": "PE_",
    "DVE": "DVE_",
    "Activation": "Activation_",
    "SP": "SP_",
}


def _strip_self_waits(nc, mybir):
    """Several TRN2 ISA instruction structs encode at most ONE sync wait
    (walrus: "Too many sync wait commands").  Two classes of Tile-emitted
    waits are redundant and safe to drop from instructions carrying >=2:

    1. Self-engine waits: an engine completes its own instructions in order.
    2. DMAHW/DMASW waits on the wq streaming loads: the slot's previous DMA
       was fully consumed by the DVE dequant before the slot-release (DVE)
       wait tick, so the DVE wait transitively covers the DMA-WAW ordering
       (Tile's per-proc vector clock does not track transitivity).
    """
    fn = nc.m.functions[0]
    observed: dict = {}
    for b in fn.blocks:
        for inst in b.instructions:
            si = inst.sync_info
            if si is None or not si.on_wait:
                continue
            eng = str(inst.engine)
            if len(si.on_wait) < 2:
                for w in si.on_wait:
                    k = (eng, w.ant_name)
                    observed[k] = max(observed.get(k, 0), w.wait_value)
                continue
            keep = [
                w
                for w in si.on_wait
                if observed.get((eng, w.ant_name), 0) < w.wait_value
            ]
            pref = _ENGINE_SEM_PREFIX.get(str(inst.engine).split(".")[-1])
            if pref is not None:
                keep = [w for w in keep if not w.ant_name.startswith(pref)]
            if len(keep) >= 2 and type(inst).__name__ == "InstDMACopy":
                if any(
                    not w.ant_name.startswith(("DMAHW", "DMASW")) for w in keep
                ):
                    keep = [
                        w
                        for w in keep
                        if not w.ant_name.startswith(("DMAHW", "DMASW"))
                    ]
            for w in keep:
                k = (eng, w.ant_name)
                observed[k] = max(observed.get(k, 0), w.wait_value)
            if len(keep) != len(si.on_wait):
                inst.sync_info = mybir.SyncInfo(
                    on_wait=keep, on_update=si.on_update
                )


def _get_nc():
    if "nc" not in _CACHE:
        _CACHE["nc"] = _build_nc()
    return _CACHE["nc"]


def _make_in_maps(x, w_q, w_scales, b_q, b_scales):
    x2 = np.ascontiguousarray(x.reshape(B, IN), dtype=np.float32)
    # xt[p, j*64+b] = x[b, j*128+p]
    xt = np.ascontiguousarray(
        x2.T.reshape(KT, 128, B).transpose(1, 0, 2).reshape(128, KT * B)
    ).astype(np.float16)
    wq_full = np.ascontiguousarray(w_q.reshape(OUT, IN))  # int32 codes
    ws_full = np.ascontiguousarray(w_scales)              # [12288, 96]
    bq_full = np.ascontiguousarray(b_q.reshape(OUT))      # int32 codes
    bs_full = np.ascontiguousarray(b_scales)              # [384]
    ident = np.eye(128, dtype=np.float16)

    in_maps = []
    for c in range(NCORES):
        o0, o1 = c * OSH, (c + 1) * OSH
        ws_c = ws_full[o0:o1]  # [1536, 96]
        swt = np.ascontiguousarray(
            ws_c.reshape(OT, 128, NB).transpose(1, 0, 2).reshape(128, OT * NB)
        )
        in_maps.append(
            {
                "wq": np.ascontiguousarray(wq_full[o0:o1]),
                "swt": swt,
                "xt": xt,
                "bq": np.ascontiguousarray(bq_full[o0:o1]).reshape(1, OSH),
                "bs": np.ascontiguousarray(
                    bs_full[o0 // BLOCK : o1 // BLOCK]
                ).reshape(1, OSH // BLOCK),
                "ident": ident,
            }
        )
    return in_maps


def run_shards(x, w_q, w_scales, b_q, b_scales, trace=False):
    """Run the SPMD kernel; returns (y_full, BassKernelResults)."""
    from concourse.bass_utils import run_bass_kernel_spmd

    nc = _get_nc()
    in_maps = _make_in_maps(x, w_q, w_scales, b_q, b_scales)
    res = run_bass_kernel_spmd(
        nc, in_maps, core_ids=list(range(NCORES)), trace=trace
    )
    shards = []
    for c in range(NCORES):
        yt = np.asarray(res.results[c]["y"])  # [128, OT*B]
        shards.append(
            yt.reshape(128, OT, B).transpose(2, 1, 0).reshape(B, OSH)
        )
    y = np.concatenate(shards, axis=1).reshape(B, 1, OUT)
    return y, res


def kernel(**inputs):
    y, _ = run_shards(
        inputs["x"],
        inputs["w_q"],
        inputs["w_scales"],
        inputs["b_q"],
        inputs["b_scales"],
        trace=False,
    )
    return y.astype(np.float32)


# revision 26
# speedup vs baseline: 1.1149x; 1.1051x over previous
"""DequantingLinear Trainium2 kernel.

y = x @ W^T + b where W = (w_q - 128) * w_scales (GGML Q8_0-style, block=32),
b = (b_q - 128) * b_scales.

Sharding: column-parallel over out_features across 8 cores (1536 rows of W
per core, 18.9 MB of int32 codes each — the HBM-bound stream).

Dataflow per core, per 128-row o-tile:
  1. w_q streams in as 12 x 1.5 MB contiguous tiles, alternating between the
     SP HWDGE ring and the GpSimd SWDGE ring so each ring's per-transfer
     completion-receipt tail (~1us) hides behind the other ring's stream.
  2. DVE dequantizes with one fused scalar_tensor_tensor per half tile:
     wp = (w_q - 128) * scales -> fp16 (scales broadcast along the free dim
     with a step-0 AP).
  3. PE transposes wp 128x128 chunks (is_transpose matmul vs identity) into
     [128,1024] fp16 PSUM banks; ACT evacuates each bank in one copy.
  4. PE accumulates yT[128, 64] = sum_j wpt_j^T-as-lhsT @ xt_j — the
     transposed W chunk is the STATIONARY operand and the (tiny) activation
     x is the moving one, so each matmul streams only N=64 rows instead of
     N=128 — half the PE matmul time of the y-form.  The bias adds via one
     K=1 matmul (bias_row ⊗ ones-vector) into the same PSUM accumulator.
  5. yT shard [128, 12*64] fp32 DMAs out; the host transposes/concatenates.

x / scales / bias / identity are host-prepared into layouts whose DMA is
contiguous per partition (no strided descriptor sprays); w_q streams through
the device untouched.

Toolchain quirks handled (see _strip_self_waits and _patch_drain_split):
every ISA instruction encodes at most ONE semaphore wait; cheap same-engine
absorber ops take the DMA/slot-release waits up front, a post-pass drops
provably redundant waits, and the kernel-tail drain's global-clock waits are
pre-spread across SP nops.
"""

import sys

import numpy as np

for _p in ("/opt/trn_rl_repo", "/root/.axon_site/_ro/trn_rl_repo"):
    if _p not in sys.path:
        sys.path.append(_p)

B = 64          # batch (x is [64, 1, 3072])
IN = 3072       # in_features
OUT = 12288     # out_features
BLOCK = 32      # quant block
NB = IN // BLOCK            # 96 blocks per row
NCORES = 8
OSH = OUT // NCORES         # 1536 out features per core
OT = OSH // 128             # 12 o-tiles of 128 rows per core
KT = IN // 128              # 24 contraction tiles
NG = 3                      # k-chunk groups of 8 per o-tile

_CACHE: dict = {}


def _patch_drain_split():
    """The TRN2 ISA gives every instruction exactly ONE inline wait slot;
    Tile's kernel-tail drain asks for the whole global clock (~11 sems) on a
    single instruction, which walrus sometimes refuses ("Too many sync wait
    commands").  Pre-spread those waits across one SP nop per semaphore; the
    drain's own waits then elide via the SP engine clock."""
    from concourse import tile as tile_mod

    if getattr(tile_mod.TileContext, "_drain_split_patched", False):
        return
    from concourse.vector_clock import ScopedClock, VectorClock

    orig = tile_mod.TileContext._drain_and_barrier

    def patched(self, tick_clock, wait_clock):
        gvc = tick_clock.global_clock
        n = len(gvc)
        for p in range(n):
            t = gvc[p]
            if t <= 0:
                continue
            vc = VectorClock([0] * n)
            vc.require_at_least(p, t)
            nop = self.nc.sync.nop(hint="drain_wait_split", nofuse=True)
            wait_clock.add_sem_waits(nop.ins, ScopedClock({None: vc}))
        return orig(self, tick_clock, wait_clock)

    tile_mod.TileContext._drain_and_barrier = patched
    tile_mod.TileContext._drain_split_patched = True


def _build_nc():
    import concourse.bass as bass
    import concourse.mybir as mybir
    from concourse.tile import TileContext
    from contextlib import ExitStack

    _patch_drain_split()

    f32 = mybir.dt.float32
    i32 = mybir.dt.int32
    f16 = mybir.dt.float16

    nc = bass.Bass()
    wq = nc.declare_dram_parameter("wq", [OSH, IN], i32, isOutput=False)
    # swt[p, t*96+k] = w_scales[t*128+p, k]  (host-prepared, contiguous DMA)
    swt = nc.declare_dram_parameter("swt", [128, OT * NB], f32, isOutput=False)
    # xt[p, j*64+b] = x[b, j*128+p]
    xt = nc.declare_dram_parameter("xt", [128, KT * B], f16, isOutput=False)
    bq = nc.declare_dram_parameter("bq", [1, OSH], i32, isOutput=False)
    bs = nc.declare_dram_parameter("bs", [1, OSH // BLOCK], f32, isOutput=False)
    ident = nc.declare_dram_parameter("ident", [128, 128], f16, isOutput=False)
    # y[p, t*64+b] = y_full[b, c*1536 + t*128 + p]
    y = nc.declare_dram_parameter("y", [128, OT * B], f32, isOutput=True)

    with TileContext(nc) as tc, ExitStack() as ctx:
        const = ctx.enter_context(tc.tile_pool(name="const", bufs=1))
        wq_pool = ctx.enter_context(tc.tile_pool(name="wq", bufs=3))
        wq1_pool = ctx.enter_context(tc.tile_pool(name="wq1", bufs=1))
        wp_pool = ctx.enter_context(tc.tile_pool(name="wp", bufs=5))
        wpt_pool = ctx.enter_context(tc.tile_pool(name="wpt", bufs=4))
        ysb_pool = ctx.enter_context(tc.tile_pool(name="ysb", bufs=1))
        pt_pool = ctx.enter_context(tc.tile_pool(name="pt", bufs=5, space="PSUM"))
        yt_pool = ctx.enter_context(tc.tile_pool(name="yt", bufs=2, space="PSUM"))
        sc_pool = ctx.enter_context(tc.tile_pool(name="sc", bufs=1, space="PSUM"))

        # --- wq stream + hot constants all on the single SP HWDGE ring (a
        # split across two rings measurably collapses aggregate HBM
        # bandwidth from ~430 to ~320 GB/s; the SWDGE const ring proved too
        # slow — scales landed at ~15us and gated the first dequant).  Ring
        # order: s_all (gates first dequant), tile 0 in group-aligned thirds
        # (earliest dequant start), identity + xt (needed by the first
        # transposes / matmuls), tile 1, then 3 MB pairs (amortize the
        # per-dma_start receipt tail; singles measurably stream ~15% slower),
        # tile 10, and tile 11 in thirds (short drain tail).
        s_all = const.tile([128, OT * NB], f32)
        nc.sync.dma_start(s_all[:], swt[:, :])
        wq_view = {}
        wq_s0 = wq1_pool.tile([128, IN], i32)
        for g in range(NG):
            sl = slice(g * 1024, (g + 1) * 1024)
            nc.sync.dma_start(wq_s0[:, sl], wq[0:128, sl])
        wq_view[0] = wq_s0
        id_sb = const.tile([128, 128], f16)
        nc.sync.dma_start(id_sb[:], ident[:, :])
        xt_sb = const.tile([128, KT * B], f16)
        nc.sync.dma_start(xt_sb[:], xt[:, :])
        wq_s1 = wq1_pool.tile([128, IN], i32)
        nc.sync.dma_start(wq_s1[:], wq[128:256, :])
        wq_view[1] = wq_s1
        for h in range(1, OT // 2 - 1):
            wq_t = wq_pool.tile([128, 2 * IN], i32)
            nc.sync.dma_start(
                wq_t[:].rearrange("p (t f) -> p t f", t=2),
                wq[256 * h : 256 * (h + 1), :].rearrange("(t p) f -> p t f", p=128),
            )
            wq_view[2 * h] = wq_t[:, 0:IN]
            wq_view[2 * h + 1] = wq_t[:, IN : 2 * IN]
        wq_s10 = wq1_pool.tile([128, IN], i32)
        nc.sync.dma_start(wq_s10[:], wq[128 * (OT - 2) : 128 * (OT - 1), :])
        wq_view[OT - 2] = wq_s10
        wq_s11 = wq1_pool.tile([128, IN], i32)
        for g in range(NG):
            sl = slice(g * 1024, (g + 1) * 1024)
            nc.sync.dma_start(
                wq_s11[:, sl], wq[128 * (OT - 1) : 128 * OT, sl]
            )
        wq_view[OT - 1] = wq_s11

        # --- cold small inputs on the GpSimd SWDGE ring (bias path only) ---
        bq_sb = const.tile([1, OSH], i32)
        nc.gpsimd.dma_start(bq_sb[:], bq[:, :])
        bs_sb = const.tile([1, OSH // BLOCK], f32)
        nc.gpsimd.dma_start(bs_sb[:], bs[:, :])

        # Wait-absorber scratch: the TensorScalarPtr(STT) ISA struct carries
        # at most ONE sync wait; before each STT we touch its input/output
        # tiles with cheap DVE ops so DMA-completion / slot-release waits
        # attach to those instead.
        scr = const.tile([1, 64], f32)
        ones = const.tile([1, B], f16)
        nc.vector.memset(ones[0:1, :], 1.0)

        # bias dequant happens per o-tile inside the loop ([1, 128] STT,
        # ~200ns) so it never blocks the in-order DVE weight-dequant stream.
        bias_sb = const.tile([1, OSH], f16)

        y_sb = ysb_pool.tile([128, OT * B], f32)

        # PE wait-absorbers: the matmul LW ISA struct also carries at most
        # one sync wait.  Touch each constant input with a tiny matmul so
        # one-time DMA waits spread over separate PE instructions.  Only
        # id/xt gate the first transposes; the bias/ones absorbers are
        # emitted later (before tile 0's bias matmul) so the bias-path DMAs
        # don't delay PE start.
        scrap = sc_pool.tile([1, 8], f32, tag="sc")
        nc.tensor.matmul(
            scrap[0:1, 0:1], id_sb[:, 0:1], id_sb[:, 0:1],
            start=True, stop=True,
        )

        for t in range(OT):
            wq_t = wq_view[t]
            wp_t = wp_pool.tile([128, IN], f16)
            if t == 0:
                # one-time absorbers for the bias-path DMA waits
                nc.vector.tensor_copy(scr[0:1, 0:1], bq_sb[0:1, 0:1])
                nc.vector.tensor_copy(scr[0:1, 1:2], bs_sb[0:1, 0:1])
            nc.vector.tensor_copy(scr[0:1, 4 + t : 5 + t], wq_t[0:1, 0:1])
            nc.vector.memset(wp_t[0:1, 0:1], 0.0)
            # per-tile bias dequant: [1, 128] slice, 4 scale blocks
            nc.vector.scalar_tensor_tensor(
                bias_sb[0:1, 128 * t : 128 * (t + 1)].rearrange(
                    "o (k j) -> o k j", j=BLOCK
                ),
                bq_sb[0:1, 128 * t : 128 * (t + 1)].rearrange(
                    "o (k j) -> o k j", j=BLOCK
                ),
                128.0,
                bs_sb[0:1, 4 * t : 4 * (t + 1)]
                .unsqueeze(2)
                .broadcast_to([1, 4, BLOCK]),
                mybir.AluOpType.subtract,
                mybir.AluOpType.mult,
            )

            yt = yt_pool.tile([128, B], f32)
            # Per k-chunk group of 8 ([128, 1024]): one dequant STT third,
            # 8 transposes into one fp16 PSUM bank, ONE ACT evacuation, then
            # 8 matmuls with the evacuated W^T chunk stationary and xt
            # moving (N=64).  Group-aligned dequant keeps the transpose /
            # evac / matmul chain chasing each dequant chunk closely.
            for g in range(NG):
                sl = slice(g * 1024, (g + 1) * 1024)
                nc.vector.scalar_tensor_tensor(
                    wp_t[:, sl].rearrange("p (k j) -> p k j", j=BLOCK),
                    wq_t[:, sl].rearrange("p (k j) -> p k j", j=BLOCK),
                    128.0,
                    s_all[:, t * NB + g * NB // NG : t * NB + (g + 1) * NB // NG]
                    .unsqueeze(2)
                    .broadcast_to([128, NB // NG, BLOCK]),
                    mybir.AluOpType.subtract,
                    mybir.AluOpType.mult,
                )
                pt = pt_pool.tile([128, 1024], f16)
                for jj in range(8):
                    j = 8 * g + jj
                    nc.tensor.transpose(
                        pt[:, 128 * jj : 128 * (jj + 1)],
                        wp_t[:, 128 * j : 128 * (j + 1)],
                        id_sb[:],
                    )
                wpt = wpt_pool.tile([128, 1024], f16)
                nc.scalar.copy(wpt[:], pt[:])
                if t == 0 and g == 0:
                    # xt wait-absorber right before its first real use (PE
                    # start must not gate on the xt DMA)
                    nc.tensor.matmul(
                        scrap[0:1, 1:2], xt_sb[:, 0:1], xt_sb[:, 0:1],
                        start=True, stop=True,
                    )
                for jj in range(8):
                    j = 8 * g + jj
                    nc.tensor.matmul(
                        yt[:],
                        wpt[:, 128 * jj : 128 * (jj + 1)],
                        xt_sb[:, B * j : B * (j + 1)],
                        start=(j == 0),
                        stop=False,
                    )
            if t == 0:
                # bias/ones PE wait-absorbers, right before first use
                for i, src in enumerate((bias_sb, ones)):
                    nc.tensor.matmul(
                        scrap[0:1, 2 + i : 3 + i], src[0:1, 0:1], src[0:1, 0:1],
                        start=True, stop=True,
                    )
            # += bias via K=1 matmul: bias row (stationary) ⊗ ones (moving)
            nc.tensor.matmul(
                yt[:],
                bias_sb[0:1, 128 * t : 128 * (t + 1)],
                ones[0:1, :],
                start=False,
                stop=True,
            )
            nc.scalar.copy(y_sb[:, B * t : B * (t + 1)], yt[:])
            if t == OT // 2 - 1:
                # first half of the output leaves early so the final DMA is
                # half as long after the last tile completes
                nc.sync.dma_start(
                    y[:, : B * OT // 2], y_sb[:, : B * OT // 2]
                )

        nc.sync.dma_start(
            y[:, B * OT // 2 :], y_sb[:, B * OT // 2 :]
        )

    _strip_self_waits(nc, mybir)
    return nc


# NOTE: Pool (GPSIMD) is deliberately absent — it is 8 parallel Q7 cores, so
# same-engine ordering does NOT hold there and its self-waits are load-bearing.
_ENGINE_SEM_PREFIX = {
    "PE": "PE_",
    "DVE": "DVE_",
    "Activation": "Activation_",
    "SP": "SP_",
}


def _strip_self_waits(nc, mybir):
    """Several TRN2 ISA instruction structs encode at most ONE sync wait
    (walrus: "Too many sync wait commands").  Two classes of Tile-emitted
    waits are redundant and safe to drop from instructions carrying >=2:

    1. Self-engine waits: an engine completes its own instructions in order.
    2. DMAHW/DMASW waits on the wq streaming loads: the slot's previous DMA
       was fully consumed by the DVE dequant before the slot-release (DVE)
       wait tick, so the DVE wait transitively covers the DMA-WAW ordering
       (Tile's per-proc vector clock does not track transitivity).
    """
    fn = nc.m.functions[0]
    observed: dict = {}
    for b in fn.blocks:
        for inst in b.instructions:
            si = inst.sync_info
            if si is None or not si.on_wait:
                continue
            eng = str(inst.engine)
            if len(si.on_wait) < 2:
                for w in si.on_wait:
                    k = (eng, w.ant_name)
                    observed[k] = max(observed.get(k, 0), w.wait_value)
                continue
            keep = [
                w
                for w in si.on_wait
                if observed.get((eng, w.ant_name), 0) < w.wait_value
            ]
            pref = _ENGINE_SEM_PREFIX.get(str(inst.engine).split(".")[-1])
            if pref is not None:
                keep = [w for w in keep if not w.ant_name.startswith(pref)]
            if len(keep) >= 2 and type(inst).__name__ == "InstDMACopy":
                if any(
                    not w.ant_name.startswith(("DMAHW", "DMASW")) for w in keep
                ):
                    keep = [
                        w
                        for w in keep
                        if not w.ant_name.startswith(("DMAHW", "DMASW"))
                    ]
            for w in keep:
                k = (eng, w.ant_name)
                observed[k] = max(observed.get(k, 0), w.wait_value)
            if len(keep) != len(si.on_wait):
                inst.sync_info = mybir.SyncInfo(
                    on_wait=keep, on_update=si.on_update
                )


def _get_nc():
    if "nc" not in _CACHE:
        _CACHE["nc"] = _build_nc()
    return _CACHE["nc"]


def _make_in_maps(x, w_q, w_scales, b_q, b_scales):
    x2 = np.ascontiguousarray(x.reshape(B, IN), dtype=np.float32)
    # xt[p, j*64+b] = x[b, j*128+p]
    xt = np.ascontiguousarray(
        x2.T.reshape(KT, 128, B).transpose(1, 0, 2).reshape(128, KT * B)
    ).astype(np.float16)
    wq_full = np.ascontiguousarray(w_q.reshape(OUT, IN))  # int32 codes
    ws_full = np.ascontiguousarray(w_scales)              # [12288, 96]
    bq_full = np.ascontiguousarray(b_q.reshape(OUT))      # int32 codes
    bs_full = np.ascontiguousarray(b_scales)              # [384]
    ident = np.eye(128, dtype=np.float16)

    in_maps = []
    for c in range(NCORES):
        o0, o1 = c * OSH, (c + 1) * OSH
        ws_c = ws_full[o0:o1]  # [1536, 96]
        swt = np.ascontiguousarray(
            ws_c.reshape(OT, 128, NB).transpose(1, 0, 2).reshape(128, OT * NB)
        )
        in_maps.append(
            {
                "wq": np.ascontiguousarray(wq_full[o0:o1]),
                "swt": swt,
                "xt": xt,
                "bq": np.ascontiguousarray(bq_full[o0:o1]).reshape(1, OSH),
                "bs": np.ascontiguousarray(
                    bs_full[o0 // BLOCK : o1 // BLOCK]
                ).reshape(1, OSH // BLOCK),
                "ident": ident,
            }
        )
    return in_maps


def run_shards(x, w_q, w_scales, b_q, b_scales, trace=False):
    """Run the SPMD kernel; returns (y_full, BassKernelResults)."""
    from concourse.bass_utils import run_bass_kernel_spmd

    nc = _get_nc()
    in_maps = _make_in_maps(x, w_q, w_scales, b_q, b_scales)
    res = run_bass_kernel_spmd(
        nc, in_maps, core_ids=list(range(NCORES)), trace=trace
    )
    shards = []
    for c in range(NCORES):
        yt = np.asarray(res.results[c]["y"])  # [128, OT*B]
        shards.append(
            yt.reshape(128, OT, B).transpose(2, 1, 0).reshape(B, OSH)
        )
    y = np.concatenate(shards, axis=1).reshape(B, 1, OUT)
    return y, res


def kernel(**inputs):
    y, _ = run_shards(
        inputs["x"],
        inputs["w_q"],
        inputs["w_scales"],
        inputs["b_q"],
        inputs["b_scales"],
        trace=False,
    )
    return y.astype(np.float32)
